# revision 1
# baseline (speedup 1.0000x reference)
"""MLPConv (3x3 valid conv -> 256 -> 256 MLP with ReLU) on 8 TRN2 cores.

Data-parallel over batch: 4 images per core. Per image, the conv is
computed as 9 PSUM-accumulated matmuls (one per filter tap) contracting
over C=128 on the partition dim, with the input transposed on the PE
(identity matmul) into [C, H*W] layout. Both MLP stages keep the
[F, pixels] transposed layout so stage-2 consumes stage-1's output
directly and the per-partition bias lands on the ACT engine's bias port.
Matmuls run as float32r (replicated fp32, 1 cycle/row at N>=256).

Output per core is [F_half, f, img, 62*64 grid]; the host slices the
valid 62 columns and assembles the [F, N, B]-ordered buffer that the
reference reinterprets as [B, 62, 62, F].
"""

import numpy as np

import concourse.bass as bass
import concourse.mybir as mybir
import concourse.tile as tile
from concourse.bass_utils import run_bass_kernel_spmd
from concourse.masks import make_identity

B, H, W, C = 32, 64, 64, 128
F = 256
N_CORES = 8
IMG_PER_CORE = B // N_CORES
HW = H * W                      # 4096 input pixels per image
GRID = 62 * 64                  # 3968 output-grid pixels (64-wide, 62 rows)
NBLK = 8
BLK = GRID // NBLK              # 496 <= 512 fp32 moving-dim limit
XT_PAD = HW + 2 * W + 2         # moving slices reach index 4097

F32 = mybir.dt.float32
F32R = mybir.dt.float32r
BF16 = mybir.dt.bfloat16
RELU = mybir.ActivationFunctionType.Relu


def _split_multi_waits(nc):
    """This container's walrus rejects >1 semaphore wait per instruction
    ("Too many sync wait commands"). Move all but the last wait of each
    instruction onto single-wait NoOps right before it on the same engine."""
    n = 0
    for f in nc.m.functions:
        for bb in f.blocks:
            insts = bb.instructions
            if not any(
                i.sync_info is not None and len(i.sync_info.on_wait) > 1
                for i in insts
            ):
                continue
            new_insts = []
            for inst in insts:
                si = inst.sync_info
                if si is not None and len(si.on_wait) > 1:
                    waits = list(si.on_wait)
                    for k, w in enumerate(waits[:-1]):
                        new_insts.append(
                            mybir.InstNoOp(
                                name=f"{inst.name}-wsplit{k}",
                                engine=inst.engine,
                                bass_nofuse=True,
                                sync_info=mybir.SyncInfo(on_wait=[w], on_update=[]),
                            )
                        )
                        n += 1
                    inst.sync_info = mybir.SyncInfo(
                        on_wait=[waits[-1]], on_update=list(si.on_update)
                    )
                new_insts.append(inst)
            bb.instructions = new_insts
    return n


def build_nc():
    nc = bass.Bass("TRN2", target_bir_lowering=False)
    x = nc.dram_tensor("x", [IMG_PER_CORE, HW, C], F32, kind="ExternalInput").ap()
    w0 = nc.dram_tensor("w0", [9 * C, F], F32, kind="ExternalInput").ap()
    b0 = nc.dram_tensor("b0", [F], F32, kind="ExternalInput").ap()
    w1 = nc.dram_tensor("w1", [F, F], F32, kind="ExternalInput").ap()
    b1 = nc.dram_tensor("b1", [F], F32, kind="ExternalInput").ap()
    out = nc.dram_tensor(
        "out", [2, 128, IMG_PER_CORE, GRID], F32, kind="ExternalOutput"
    ).ap()

    with tile.TileContext(nc) as tc:
        with (
            tc.tile_pool(name="consts", bufs=1) as consts,
            tc.tile_pool(name="xl", bufs=6) as xl,
            tc.tile_pool(name="xlb", bufs=6) as xlbp,
            tc.tile_pool(name="xT", bufs=2) as xT,  # two half-image tiles per img
            tc.tile_pool(name="h1T", bufs=4) as h1T,
            tc.tile_pool(name="outb", bufs=4) as outb,
            tc.tile_pool(name="pt", bufs=2, space="PSUM") as pt,
            tc.tile_pool(name="ps1", bufs=4, space="PSUM") as ps1,
            tc.tile_pool(name="ps2", bufs=2, space="PSUM") as ps2,
        ):
            ident = consts.tile([128, 128], BF16)
            make_identity(nc, ident)

            # first image's input DMAs go first so the PE can start promptly
            xls = {}
            for j in range(4):
                xlt = xl.tile([128, 8, 128], F32, name="xlt")
                nc.sync.dma_start(
                    xlt[:], x[0].rearrange("(b p) c -> p b c", p=128)[:, 8 * j : 8 * (j + 1), :]
                )
                xls[(0, j)] = xlt

            w0f = consts.tile([128, 9, F], F32)
            nc.sync.dma_start(w0f[:], w0.rearrange("(t c) f -> c t f", c=128))
            w0b = consts.tile([128, 9, F], BF16)
            nc.vector.tensor_copy(w0b[:], w0f[:])
            w1f = consts.tile([128, 2, F], F32)
            nc.sync.dma_start(w1f[:], w1.rearrange("(k c) f -> c k f", c=128))
            w1s = consts.tile([128, 2, F], F32R)
            nc.vector.tensor_copy(w1s[:], w1f[:])
            b0s = consts.tile([128, 2], F32)
            nc.sync.dma_start(b0s[:], b0.rearrange("(h f) -> f h", f=128))
            b1s = consts.tile([128, 2], F32)
            nc.sync.dma_start(b1s[:], b1.rearrange("(h f) -> f h", f=128))

            def load_and_transpose(img):
                ximg = x[img].rearrange("(b p) c -> p b c", p=128)
                xTa = xT.tile([128, 17 * 128], BF16, name="xTa")
                xTb = xT.tile([128, 18 * 128], BF16, name="xTb")  # px 1920.. + pad
                xlbs = []
                dmae = [nc.scalar, nc.gpsimd, nc.scalar, nc.gpsimd]
                for j in range(4):
                    if (img, j) in xls:
                        xlt = xls.pop((img, j))
                    else:
                        xlt = xl.tile([128, 8, 128], F32, name="xlt")
                        dmae[j].dma_start(
                            xlt[:], ximg[:, 8 * j : 8 * (j + 1), :]
                        )
                    xlb = xlbp.tile([128, 8, 128], BF16, name="xlb")
                    nc.vector.tensor_copy(xlb[:], xlt[:])
                    xlbs.append(xlb)
                for dst, p0, plist in (
                    (xTa, 0, range(0, 17)),
                    (xTb, 15, range(15, 32)),
                ):
                    for b0i in range(0, 17, 4):
                        batch = list(plist)[b0i : b0i + 4]
                        nb = len(batch)
                        ptt = pt.tile([128, 4, 128], BF16, name="ptt")
                        for q, p in enumerate(batch):
                            nc.tensor.transpose(
                                ptt[:, q, :], xlbs[p // 8][:, p % 8, :], ident[:]
                            )
                        nc.vector.tensor_copy(
                            dst[:, 128 * (batch[0] - p0) : 128 * (batch[0] - p0 + nb)],
                            ptt[:, :nb, :].rearrange("p a b -> p (a b)"),
                        )
                return xTa, xTb

            def stage1(xTa, xTb):
                h1 = []
                for h in range(2):
                    h1t = h1T.tile([128, GRID], F32R, name="h1t")
                    h1.append(h1t)
                for part, xpart, base in ((0, xTa, 0), (1, xTb, 1920)):
                    for h in range(2):
                        for g in (0, 1) if part == 0 else (2, 3):
                            pss = []
                            for bi in range(2):
                                ps1t = ps1.tile([128, BLK], F32, name="ps1t")
                                pss.append(ps1t)
                            for t in range(9):
                                off = (t // 3) * W + (t % 3)
                                wtap = w0b[:, t, 128 * h : 128 * (h + 1)]
                                for bi in range(2):
                                    s = (2 * g + bi) * BLK + off - base
                                    nc.tensor.matmul(
                                        pss[bi][:],
                                        wtap,
                                        xpart[:, s : s + BLK],
                                        start=(t == 0),
                                        stop=(t == 8),
                                    )
                            for bi in range(2):
                                s = (2 * g + bi) * BLK
                                nc.scalar.activation(
                                    h1[h][:, s : s + BLK],
                                    pss[bi][:],
                                    RELU,
                                    bias=b0s[:, h : h + 1],
                                )
                return h1

            def stage2(img, h1):
                for h in range(2):
                    for blk in range(NBLK):
                        s = blk * BLK
                        ps2t = ps2.tile([128, BLK], F32)
                        for k in range(2):
                            nc.tensor.matmul(
                                ps2t[:],
                                w1s[:, k, 128 * h : 128 * (h + 1)],
                                h1[k][:, s : s + BLK],
                                start=(k == 0),
                                stop=(k == 1),
                            )
                        ot = outb.tile([128, BLK], F32)
                        nc.scalar.activation(
                            ot[:], ps2t[:], RELU, bias=b1s[:, h : h + 1]
                        )
                        nc.sync.dma_start(out[h, :, img, s : s + BLK], ot[:])

            # Software pipeline: img i+1's transposes sit between stage1(i)
            # and stage2(i) in the PE stream, hiding the copy/cast latency.
            xab = load_and_transpose(0)
            h1_cur = stage1(*xab)
            for i in range(IMG_PER_CORE):
                if i + 1 < IMG_PER_CORE:
                    xab_next = load_and_transpose(i + 1)
                stage2(i, h1_cur)
                if i + 1 < IMG_PER_CORE:
                    h1_cur = stage1(*xab_next)

    _split_multi_waits(nc)
    return nc


_NC_CACHE = None


def kernel(inputs, w0, b0, w1, b1):
    global _NC_CACHE
    x = np.ascontiguousarray(np.asarray(inputs, dtype=np.float32))
    w0 = np.ascontiguousarray(np.asarray(w0, dtype=np.float32))
    w1 = np.ascontiguousarray(np.asarray(w1, dtype=np.float32))
    b0 = np.ascontiguousarray(np.asarray(b0, dtype=np.float32))
    b1 = np.ascontiguousarray(np.asarray(b1, dtype=np.float32))

    if _NC_CACHE is None:
        _NC_CACHE = build_nc()
    nc = _NC_CACHE

    in_maps = [
        {
            "x": x[c * IMG_PER_CORE : (c + 1) * IMG_PER_CORE].reshape(
                IMG_PER_CORE, HW, C
            ),
            "w0": w0,
            "b0": b0,
            "w1": w1,
            "b1": b1,
        }
        for c in range(N_CORES)
    ]
    res = run_bass_kernel_spmd(nc, in_maps, core_ids=list(range(N_CORES)))

    final = np.empty((B, 62, 62, F), np.float32)
    vf = final.reshape(F, 62 * 62, B)  # the [F, N, B] view the reference reshapes
    for c in range(N_CORES):
        oc = res.results[c]["out"].reshape(F, IMG_PER_CORE, 62, 64)
        oc = oc[:, :, :, :62].reshape(F, IMG_PER_CORE, 62 * 62)
        for i in range(IMG_PER_CORE):
            vf[:, :, c * IMG_PER_CORE + i] = oc[:, i]
    return final



# revision 5
# speedup vs baseline: 1.0233x; 1.0233x over previous
"""MLPConv (3x3 valid conv -> 256 -> 256 MLP with ReLU) on 8 TRN2 cores.

Data-parallel over batch: 4 images per core. The host pre-transposes each
image to [C=128, pixels] bf16 (zero-padded to 4104 cols) so the device does
no transposes, casts, or copies: the PE runs a pure matmul stream. The conv
is 9 PSUM-accumulated matmuls per 496-pixel block (one per filter tap,
shifted views of the same transposed image); both MLP stages keep the
[F, pixels] layout so stage 2 consumes stage 1's bf16 output directly and
the per-partition bias + ReLU land on the ACT engine.

A burst of warmup matmuls on a memset tile runs during the input DMA to
flip the PE's HAM clock gate to 2.4 GHz before real work starts (the
baseline spent its first ~42us at 1.2 GHz).

Output per core is [2, 128, img, 62*64 grid]; the host slices the valid 62
columns and assembles the [F, N, B]-ordered buffer that the reference
reinterprets as [B, 62, 62, F].
"""

import numpy as np
import ml_dtypes

import concourse.bass as bass
import concourse.mybir as mybir
import concourse.tile as tile
from concourse.bass_utils import run_bass_kernel_spmd

B, H, W, C = 32, 64, 64, 128
F = 256
N_CORES = 8
IMG_PER_CORE = B // N_CORES
HW = H * W                      # 4096 input pixels per image
GRID = 62 * 64                  # 3968 output-grid pixels (64-wide, 62 rows)
NBLK = 8
BLK = GRID // NBLK              # 496 <= 512 fp32 PSUM-bank limit
XCOLS = HW + 8                  # moving slices reach index 4097; pad w/ zeros

F32 = mybir.dt.float32
BF16 = mybir.dt.bfloat16
RELU = mybir.ActivationFunctionType.Relu


def _split_multi_waits(nc):
    """This container's walrus rejects >1 semaphore wait per instruction
    ("Too many sync wait commands"). Move all but the last wait of each
    instruction onto single-wait NoOps right before it on the same engine."""
    n = 0
    for f in nc.m.functions:
        for bb in f.blocks:
            insts = bb.instructions
            if not any(
                i.sync_info is not None and len(i.sync_info.on_wait) > 1
                for i in insts
            ):
                continue
            new_insts = []
            for inst in insts:
                si = inst.sync_info
                if si is not None and len(si.on_wait) > 1:
                    waits = list(si.on_wait)
                    for k, w in enumerate(waits[:-1]):
                        new_insts.append(
                            mybir.InstNoOp(
                                name=f"{inst.name}-wsplit{k}",
                                engine=inst.engine,
                                bass_nofuse=True,
                                sync_info=mybir.SyncInfo(on_wait=[w], on_update=[]),
                            )
                        )
                        n += 1
                    inst.sync_info = mybir.SyncInfo(
                        on_wait=[waits[-1]], on_update=list(si.on_update)
                    )
                new_insts.append(inst)
            bb.instructions = new_insts
    return n


def build_nc():
    nc = bass.Bass("TRN2", target_bir_lowering=False)
    x = nc.dram_tensor(
        "x", [IMG_PER_CORE, C, XCOLS], BF16, kind="ExternalInput"
    ).ap()
    w0 = nc.dram_tensor("w0", [C, 9, F], BF16, kind="ExternalInput").ap()
    w1 = nc.dram_tensor("w1", [C, 2, F], BF16, kind="ExternalInput").ap()
    b0 = nc.dram_tensor("b0", [128, 2], F32, kind="ExternalInput").ap()
    b1 = nc.dram_tensor("b1", [128, 2], F32, kind="ExternalInput").ap()
    out = nc.dram_tensor(
        "out", [2, 128, IMG_PER_CORE, GRID], F32, kind="ExternalOutput"
    ).ap()

    with tile.TileContext(nc) as tc:
        with (
            tc.tile_pool(name="consts", bufs=1) as consts,
            tc.tile_pool(name="xT", bufs=IMG_PER_CORE) as xT,
            tc.tile_pool(name="h1", bufs=4) as h1p,
            tc.tile_pool(name="outb", bufs=6) as outb,
            tc.tile_pool(name="ps1", bufs=4, space="PSUM") as ps1,
            tc.tile_pool(name="ps2", bufs=4, space="PSUM") as ps2,
        ):
            # PE warmup during input DMA: ~10 matmuls on a memset tile flip
            # the HAM clock gate to 8/8 before stage 1's first real matmul.
            warm = consts.tile([128, BLK], BF16)
            nc.gpsimd.memset(warm[:], 0.0)
            pw = ps1.tile([128, BLK], F32, name="ps1t")
            for _ in range(10):
                nc.tensor.matmul(pw[:], warm[:, :128], warm[:], start=True, stop=True)

            # x image 0 first (gates stage 1), then the rest; all on the sync
            # ring so image 0's transfer completes first.
            xts = []
            for img in range(IMG_PER_CORE):
                xt = xT.tile([128, XCOLS], BF16, name="xt")
                nc.sync.dma_start(xt[:], x[img])
                xts.append(xt)
            # weights/biases on other rings, concurrent with x image 0
            w0b = consts.tile([128, 9, F], BF16)
            nc.scalar.dma_start(w0b[:], w0)
            w1b = consts.tile([128, 2, F], BF16)
            nc.gpsimd.dma_start(w1b[:], w1)
            b0s = consts.tile([128, 2], F32)
            nc.gpsimd.dma_start(b0s[:], b0)
            b1s = consts.tile([128, 2], F32)
            nc.gpsimd.dma_start(b1s[:], b1)

            def stage1(img):
                xt = xts[img]
                h1 = []
                for k in range(2):
                    h1.append(h1p.tile([128, GRID], BF16, name="h1t"))
                for h in range(2):
                    for gp in range(NBLK // 2):
                        pss = [
                            ps1.tile([128, BLK], F32, name="ps1t") for _ in range(2)
                        ]
                        for t in range(9):
                            off = (t // 3) * W + (t % 3)
                            wtap = w0b[:, t, 128 * h : 128 * (h + 1)]
                            for bi in range(2):
                                s = (2 * gp + bi) * BLK + off
                                nc.tensor.matmul(
                                    pss[bi][:],
                                    wtap,
                                    xt[:, s : s + BLK],
                                    start=(t == 0),
                                    stop=(t == 8),
                                )
                        for bi in range(2):
                            s = (2 * gp + bi) * BLK
                            nc.scalar.activation(
                                h1[h][:, s : s + BLK],
                                pss[bi][:],
                                RELU,
                                bias=b0s[:, h : h + 1],
                            )
                return h1

            dmae = [nc.gpsimd, nc.sync]

            def stage2(img, h1):
                # k-outer over 4-block groups: one LDWEIGHTS per 4 matmuls
                for h in range(2):
                    for gq in range(2):
                        pss = [
                            ps2.tile([128, BLK], F32, name="ps2t") for _ in range(4)
                        ]
                        for k in range(2):
                            wk = w1b[:, k, 128 * h : 128 * (h + 1)]
                            for bi in range(4):
                                s = (4 * gq + bi) * BLK
                                nc.tensor.matmul(
                                    pss[bi][:],
                                    wk,
                                    h1[k][:, s : s + BLK],
                                    start=(k == 0),
                                    stop=(k == 1),
                                )
                        for bi in range(4):
                            blk = 4 * gq + bi
                            s = blk * BLK
                            ot = outb.tile([128, BLK], F32)
                            nc.scalar.activation(
                                ot[:], pss[bi][:], RELU, bias=b1s[:, h : h + 1]
                            )
                            dmae[blk % len(dmae)].dma_start(
                                out[h, :, img, s : s + BLK], ot[:]
                            )

            for img in range(IMG_PER_CORE):
                h1 = stage1(img)
                stage2(img, h1)

    _split_multi_waits(nc)
    return nc


_NC_CACHE = None


def kernel(inputs, w0, b0, w1, b1):
    global _NC_CACHE
    x = np.asarray(inputs, dtype=np.float32)
    w0 = np.asarray(w0, dtype=np.float32)
    w1 = np.asarray(w1, dtype=np.float32)
    b0 = np.asarray(b0, dtype=np.float32)
    b1 = np.asarray(b1, dtype=np.float32)

    if _NC_CACHE is None:
        _NC_CACHE = build_nc()
    nc = _NC_CACHE

    bf = ml_dtypes.bfloat16
    # per-core x: [IMG, C, XCOLS] bf16, image transposed to [C, pixels]
    xs = x.reshape(N_CORES, IMG_PER_CORE, HW, C)
    w0h = np.ascontiguousarray(
        w0.reshape(9, C, F).transpose(1, 0, 2).astype(bf)
    )
    w1h = np.ascontiguousarray(
        w1.reshape(2, C, F).transpose(1, 0, 2).astype(bf)
    )
    b0h = np.ascontiguousarray(b0.reshape(2, 128).T)
    b1h = np.ascontiguousarray(b1.reshape(2, 128).T)

    in_maps = []
    for c in range(N_CORES):
        xt = np.zeros((IMG_PER_CORE, C, XCOLS), bf)
        xt[:, :, :HW] = xs[c].transpose(0, 2, 1).astype(bf)
        in_maps.append({"x": xt, "w0": w0h, "w1": w1h, "b0": b0h, "b1": b1h})

    res = run_bass_kernel_spmd(nc, in_maps, core_ids=list(range(N_CORES)))

    final = np.empty((B, 62, 62, F), np.float32)
    vf = final.reshape(F, 62 * 62, B)  # the [F, N, B] view the reference reshapes
    for c in range(N_CORES):
        oc = res.results[c]["out"].reshape(F, IMG_PER_CORE, 62, 64)
        oc = oc[:, :, :, :62].reshape(F, IMG_PER_CORE, 62 * 62)
        for i in range(IMG_PER_CORE):
            vf[:, :, c * IMG_PER_CORE + i] = oc[:, i]
    return final


# revision 7
# speedup vs baseline: 1.0993x; 1.0743x over previous
"""MLPConv (3x3 valid conv -> 256 -> 256 MLP with ReLU) on 8 TRN2 cores.

Data-parallel over batch: 4 images per core. The host pre-transposes each
image to [C=128, pixels] bf16 (zero-padded to 4104 cols) so the device does
no transposes, casts, or copies: the PE runs a pure matmul stream. The conv
is 9 PSUM-accumulated matmuls per 496-pixel block (one per filter tap,
shifted views of the same transposed image); both MLP stages keep the
[F, pixels] layout so stage 2 consumes stage 1's bf16 output directly and
the per-partition bias + ReLU land on the ACT engine.

A burst of warmup matmuls on a memset tile runs during the input DMA to
flip the PE's HAM clock gate to 2.4 GHz before real work starts (the
baseline spent its first ~42us at 1.2 GHz).

Output per core is [2, 128, img, 62*64 grid]; the host slices the valid 62
columns and assembles the [F, N, B]-ordered buffer that the reference
reinterprets as [B, 62, 62, F].
"""

import numpy as np
import ml_dtypes

import concourse.bass as bass
import concourse.mybir as mybir
import concourse.tile as tile
from concourse.bass_utils import run_bass_kernel_spmd

B, H, W, C = 32, 64, 64, 128
F = 256
N_CORES = 8
IMG_PER_CORE = B // N_CORES
HW = H * W                      # 4096 input pixels per image
GRID = 62 * 64                  # 3968 output-grid pixels (64-wide, 62 rows)
NBLK = 8
BLK = GRID // NBLK              # 496 <= 512 fp32 PSUM-bank limit
XCOLS = HW + 8                  # moving slices reach index 4097; pad w/ zeros

F32 = mybir.dt.float32
BF16 = mybir.dt.bfloat16
RELU = mybir.ActivationFunctionType.Relu


def _split_multi_waits(nc):
    """This container's walrus rejects >1 semaphore wait per instruction
    ("Too many sync wait commands"). Move all but the last wait of each
    instruction onto single-wait NoOps right before it on the same engine."""
    n = 0
    for f in nc.m.functions:
        for bb in f.blocks:
            insts = bb.instructions
            if not any(
                i.sync_info is not None and len(i.sync_info.on_wait) > 1
                for i in insts
            ):
                continue
            new_insts = []
            for inst in insts:
                si = inst.sync_info
                if si is not None and len(si.on_wait) > 1:
                    waits = list(si.on_wait)
                    for k, w in enumerate(waits[:-1]):
                        new_insts.append(
                            mybir.InstNoOp(
                                name=f"{inst.name}-wsplit{k}",
                                engine=inst.engine,
                                bass_nofuse=True,
                                sync_info=mybir.SyncInfo(on_wait=[w], on_update=[]),
                            )
                        )
                        n += 1
                    inst.sync_info = mybir.SyncInfo(
                        on_wait=[waits[-1]], on_update=list(si.on_update)
                    )
                new_insts.append(inst)
            bb.instructions = new_insts
    return n


def build_nc():
    nc = bass.Bass("TRN2", target_bir_lowering=False)
    x = nc.dram_tensor(
        "x", [IMG_PER_CORE, C, XCOLS], BF16, kind="ExternalInput"
    ).ap()
    w0 = nc.dram_tensor("w0", [C, 9, F], BF16, kind="ExternalInput").ap()
    w1 = nc.dram_tensor("w1", [C, 2, F], BF16, kind="ExternalInput").ap()
    b0 = nc.dram_tensor("b0", [128, 2], F32, kind="ExternalInput").ap()
    b1 = nc.dram_tensor("b1", [128, 2], F32, kind="ExternalInput").ap()
    out = nc.dram_tensor(
        "out", [2, 128, IMG_PER_CORE, GRID], F32, kind="ExternalOutput"
    ).ap()

    with tile.TileContext(nc) as tc:
        with (
            tc.tile_pool(name="consts", bufs=1) as consts,
            tc.tile_pool(name="xT", bufs=2) as xT,
            tc.tile_pool(name="h1", bufs=4) as h1p,
            tc.tile_pool(name="outb", bufs=6) as outb,
            tc.tile_pool(name="ps1", bufs=4, space="PSUM") as ps1,
            tc.tile_pool(name="ps2", bufs=4, space="PSUM") as ps2,
        ):
            # Weights first (smallest critical transfer), on the scalar ring.
            w0b = consts.tile([128, 9, F], BF16)
            nc.scalar.dma_start(w0b[:], w0)
            w1b = consts.tile([128, 2, F], BF16)
            nc.scalar.dma_start(w1b[:], w1)
            b0s = consts.tile([128, 2], F32)
            nc.scalar.dma_start(b0s[:], b0)
            b1s = consts.tile([128, 2], F32)
            nc.scalar.dma_start(b1s[:], b1)

            # PE warmup during input DMA: matmuls on a memset tile flip the
            # HAM clock gate to 8/8 before stage 1's first real matmul.
            warm = consts.tile([128, BLK], BF16)
            nc.gpsimd.memset(warm[:], 0.0)
            pws = [ps1.tile([128, BLK], F32, name="ps1t") for _ in range(2)]
            for i in range(12):
                nc.tensor.matmul(
                    pws[i % 2][:], warm[:, :128], warm[:], start=True, stop=True
                )

            # x loads: two chunks per image, all on the sync ring, in image
            # order — the HWDGE ring is FIFO, so image 0 completes first.
            # bufs=2 makes images 2-3 wait for stage1 of images 0-1 (the DMA
            # trigger inherits the pool-slot dependency), keeping the early
            # HBM bandwidth for the critical image-0 + weights transfers.
            XHALF = XCOLS // 2
            def load_x(img):
                xt = xT.tile([128, XCOLS], BF16, name="xt")
                nc.sync.dma_start(xt[:, :XHALF], x[img, :, :XHALF])
                nc.sync.dma_start(xt[:, XHALF:], x[img, :, XHALF:])
                return xt

            xts = [load_x(0), load_x(1)]

            def stage1(img):
                xt = xts[img]
                h1 = []
                for k in range(2):
                    h1.append(h1p.tile([128, GRID], BF16, name="h1t"))
                for h in range(2):
                    for gp in range(NBLK // 2):
                        pss = [
                            ps1.tile([128, BLK], F32, name="ps1t") for _ in range(2)
                        ]
                        for t in range(9):
                            off = (t // 3) * W + (t % 3)
                            wtap = w0b[:, t, 128 * h : 128 * (h + 1)]
                            for bi in range(2):
                                s = (2 * gp + bi) * BLK + off
                                nc.tensor.matmul(
                                    pss[bi][:],
                                    wtap,
                                    xt[:, s : s + BLK],
                                    start=(t == 0),
                                    stop=(t == 8),
                                )
                        for bi in range(2):
                            s = (2 * gp + bi) * BLK
                            nc.scalar.activation(
                                h1[h][:, s : s + BLK],
                                pss[bi][:],
                                RELU,
                                bias=b0s[:, h : h + 1],
                            )
                return h1

            dmae = [nc.gpsimd, nc.sync]

            def stage2(img, h1):
                # k-outer over 4-block groups: one LDWEIGHTS per 4 matmuls
                for h in range(2):
                    for gq in range(2):
                        pss = [
                            ps2.tile([128, BLK], F32, name="ps2t") for _ in range(4)
                        ]
                        for k in range(2):
                            wk = w1b[:, k, 128 * h : 128 * (h + 1)]
                            for bi in range(4):
                                s = (4 * gq + bi) * BLK
                                nc.tensor.matmul(
                                    pss[bi][:],
                                    wk,
                                    h1[k][:, s : s + BLK],
                                    start=(k == 0),
                                    stop=(k == 1),
                                )
                        for bi in range(4):
                            blk = 4 * gq + bi
                            s = blk * BLK
                            ot = outb.tile([128, BLK], F32)
                            nc.scalar.activation(
                                ot[:], pss[bi][:], RELU, bias=b1s[:, h : h + 1]
                            )
                            dmae[blk % len(dmae)].dma_start(
                                out[h, :, img, s : s + BLK], ot[:]
                            )

            for img in range(IMG_PER_CORE):
                h1 = stage1(img)
                stage2(img, h1)

    _split_multi_waits(nc)
    return nc


_NC_CACHE = None


def kernel(inputs, w0, b0, w1, b1):
    global _NC_CACHE
    x = np.asarray(inputs, dtype=np.float32)
    w0 = np.asarray(w0, dtype=np.float32)
    w1 = np.asarray(w1, dtype=np.float32)
    b0 = np.asarray(b0, dtype=np.float32)
    b1 = np.asarray(b1, dtype=np.float32)

    if _NC_CACHE is None:
        _NC_CACHE = build_nc()
    nc = _NC_CACHE

    bf = ml_dtypes.bfloat16
    # per-core x: [IMG, C, XCOLS] bf16, image transposed to [C, pixels]
    xs = x.reshape(N_CORES, IMG_PER_CORE, HW, C)
    w0h = np.ascontiguousarray(
        w0.reshape(9, C, F).transpose(1, 0, 2).astype(bf)
    )
    w1h = np.ascontiguousarray(
        w1.reshape(2, C, F).transpose(1, 0, 2).astype(bf)
    )
    b0h = np.ascontiguousarray(b0.reshape(2, 128).T)
    b1h = np.ascontiguousarray(b1.reshape(2, 128).T)

    in_maps = []
    for c in range(N_CORES):
        xt = np.zeros((IMG_PER_CORE, C, XCOLS), bf)
        xt[:, :, :HW] = xs[c].transpose(0, 2, 1).astype(bf)
        in_maps.append({"x": xt, "w0": w0h, "w1": w1h, "b0": b0h, "b1": b1h})

    res = run_bass_kernel_spmd(nc, in_maps, core_ids=list(range(N_CORES)))

    final = np.empty((B, 62, 62, F), np.float32)
    vf = final.reshape(F, 62 * 62, B)  # the [F, N, B] view the reference reshapes
    for c in range(N_CORES):
        oc = res.results[c]["out"].reshape(F, IMG_PER_CORE, 62, 64)
        oc = oc[:, :, :, :62].reshape(F, IMG_PER_CORE, 62 * 62)
        for i in range(IMG_PER_CORE):
            vf[:, :, c * IMG_PER_CORE + i] = oc[:, i]
    return final


# revision 10
# speedup vs baseline: 1.2048x; 1.0960x over previous
"""MLPConv (3x3 valid conv -> 256 -> 256 MLP with ReLU) on 8 TRN2 cores.

Data-parallel over batch: 4 images per core. The host pre-transposes each
image to [C=128, pixels] bf16 (zero-padded to 4104 cols) so the device does
no transposes, casts, or copies: the PE runs a pure matmul stream. The conv
is 9 PSUM-accumulated matmuls per 496-pixel block (one per filter tap,
shifted views of the same transposed image); both MLP stages keep the
[F, pixels] layout so stage 2 consumes stage 1's bf16 output directly and
the per-partition bias + ReLU land on the ACT engine.

A burst of warmup matmuls on a memset tile runs during the input DMA to
flip the PE's HAM clock gate to 2.4 GHz before real work starts (the
baseline spent its first ~42us at 1.2 GHz).

Output per core is [2, 128, img, 62*64 grid]; the host slices the valid 62
columns and assembles the [F, N, B]-ordered buffer that the reference
reinterprets as [B, 62, 62, F].
"""

import numpy as np
import ml_dtypes

import concourse.bass as bass
import concourse.mybir as mybir
import concourse.tile as tile
from concourse.bass_utils import run_bass_kernel_spmd

B, H, W, C = 32, 64, 64, 128
F = 256
N_CORES = 8
IMG_PER_CORE = B // N_CORES
HW = H * W                      # 4096 input pixels per image
GRID = 62 * 64                  # 3968 output-grid pixels (64-wide, 62 rows)
NBLK = 8
BLK = GRID // NBLK              # 496 <= 512 fp32 PSUM-bank limit
XCOLS = HW + 8                  # moving slices reach index 4097; pad w/ zeros

F32 = mybir.dt.float32
BF16 = mybir.dt.bfloat16
RELU = mybir.ActivationFunctionType.Relu


def _split_multi_waits(nc):
    """This container's walrus rejects >1 semaphore wait per instruction
    ("Too many sync wait commands"). Move all but the last wait of each
    instruction onto single-wait NoOps right before it on the same engine."""
    n = 0
    for f in nc.m.functions:
        for bb in f.blocks:
            insts = bb.instructions
            if not any(
                i.sync_info is not None and len(i.sync_info.on_wait) > 1
                for i in insts
            ):
                continue
            new_insts = []
            for inst in insts:
                si = inst.sync_info
                if si is not None and len(si.on_wait) > 1:
                    waits = list(si.on_wait)
                    for k, w in enumerate(waits[:-1]):
                        new_insts.append(
                            mybir.InstNoOp(
                                name=f"{inst.name}-wsplit{k}",
                                engine=inst.engine,
                                bass_nofuse=True,
                                sync_info=mybir.SyncInfo(on_wait=[w], on_update=[]),
                            )
                        )
                        n += 1
                    inst.sync_info = mybir.SyncInfo(
                        on_wait=[waits[-1]], on_update=list(si.on_update)
                    )
                new_insts.append(inst)
            bb.instructions = new_insts
    return n


def build_nc():
    nc = bass.Bass("TRN2", target_bir_lowering=False)
    x = nc.dram_tensor(
        "x", [IMG_PER_CORE, C, XCOLS], BF16, kind="ExternalInput"
    ).ap()
    w0 = nc.dram_tensor("w0", [C, 9, F], BF16, kind="ExternalInput").ap()
    w1 = nc.dram_tensor("w1", [C, 2, F], BF16, kind="ExternalInput").ap()
    b0 = nc.dram_tensor("b0", [128, 2], F32, kind="ExternalInput").ap()
    b1 = nc.dram_tensor("b1", [128, 2], F32, kind="ExternalInput").ap()
    out = nc.dram_tensor(
        "out", [2, 128, IMG_PER_CORE, GRID], F32, kind="ExternalOutput"
    ).ap()

    with tile.TileContext(nc) as tc:
        with (
            tc.tile_pool(name="consts", bufs=1) as consts,
            tc.tile_pool(name="xT", bufs=2) as xT,
            tc.tile_pool(name="h1", bufs=4) as h1p,
            tc.tile_pool(name="outb", bufs=6) as outb,
            tc.tile_pool(name="ps1", bufs=4, space="PSUM") as ps1,
            tc.tile_pool(name="ps2", bufs=4, space="PSUM") as ps2,
        ):
            # Weights first (smallest critical transfer), on the scalar ring.
            w0b = consts.tile([128, 9, F], BF16)
            nc.scalar.dma_start(w0b[:], w0)
            w1b = consts.tile([128, 2, F], BF16)
            nc.scalar.dma_start(w1b[:], w1)
            b0s = consts.tile([128, 2], F32)
            nc.scalar.dma_start(b0s[:], b0)
            b1s = consts.tile([128, 2], F32)
            nc.scalar.dma_start(b1s[:], b1)

            # PE warmup during input DMA: matmuls on a memset tile flip the
            # HAM clock gate to 8/8 before stage 1's first real matmul.
            warm = consts.tile([128, BLK], BF16)
            nc.gpsimd.memset(warm[:], 0.0)
            pws = [ps1.tile([128, BLK], F32, name="ps1t") for _ in range(2)]
            for i in range(12):
                nc.tensor.matmul(
                    pws[i % 2][:], warm[:, :128], warm[:], start=True, stop=True
                )

            # x loads: two chunks per image, all on the sync ring, in image
            # order — the HWDGE ring is FIFO, so image 0 completes first.
            # bufs=2 makes images 2-3 wait for stage1 of images 0-1 (the DMA
            # trigger inherits the pool-slot dependency), keeping the early
            # HBM bandwidth for the critical image-0 + weights transfers.
            XHALF = XCOLS // 2
            def load_x(img):
                xt = xT.tile([128, XCOLS], BF16, name="xt")
                nc.sync.dma_start(xt[:, :XHALF], x[img, :, :XHALF])
                nc.sync.dma_start(xt[:, XHALF:], x[img, :, XHALF:])
                return xt

            xts = [load_x(0), load_x(1)]

            def stage1(img):
                xt = xts[img]
                h1 = []
                for k in range(2):
                    h1.append(h1p.tile([128, GRID], BF16, name="h1t"))
                for h in range(2):
                    for gp in range(NBLK // 2):
                        pss = [
                            ps1.tile([128, BLK], F32, name="ps1t") for _ in range(2)
                        ]
                        for t in range(9):
                            off = (t // 3) * W + (t % 3)
                            wtap = w0b[:, t, 128 * h : 128 * (h + 1)]
                            for bi in range(2):
                                s = (2 * gp + bi) * BLK + off
                                nc.tensor.matmul(
                                    pss[bi][:],
                                    wtap,
                                    xt[:, s : s + BLK],
                                    start=(t == 0),
                                    stop=(t == 8),
                                )
                        for bi in range(2):
                            s = (2 * gp + bi) * BLK
                            nc.scalar.activation(
                                h1[h][:, s : s + BLK],
                                pss[bi][:],
                                RELU,
                                bias=b0s[:, h : h + 1],
                            )
                return h1

            ADD = mybir.AluOpType.add
            MAX = mybir.AluOpType.max

            def stage2(img, h1):
                # k-outer over 4-block groups: one LDWEIGHTS per 4 matmuls.
                # Post-processing alternates scalar-ACT and DVE (both compute
                # relu(psum + b1)) so neither engine is the bottleneck, and
                # output DMAs batch two blocks per trigger.
                for h in range(2):
                    for gq in range(2):
                        pss = [
                            ps2.tile([128, BLK], F32, name="ps2t") for _ in range(4)
                        ]
                        for k in range(2):
                            wk = w1b[:, k, 128 * h : 128 * (h + 1)]
                            for bi in range(4):
                                s = (4 * gq + bi) * BLK
                                nc.tensor.matmul(
                                    pss[bi][:],
                                    wk,
                                    h1[k][:, s : s + BLK],
                                    start=(k == 0),
                                    stop=(k == 1),
                                )
                        ots = [
                            outb.tile([128, 2 * BLK], F32, name="ot")
                            for _ in range(2)
                        ]
                        for bi in range(4):
                            dst = ots[bi // 2][:, (bi % 2) * BLK :][:, :BLK]
                            if bi % 2 == 0:
                                nc.scalar.activation(
                                    dst, pss[bi][:], RELU, bias=b1s[:, h : h + 1]
                                )
                            else:
                                nc.vector.tensor_scalar(
                                    dst, pss[bi][:], b1s[:, h : h + 1], 0.0, ADD, MAX
                                )
                        for q in range(2):
                            s = (4 * gq + 2 * q) * BLK
                            nc.gpsimd.dma_start(
                                out[h, :, img, s : s + 2 * BLK], ots[q][:]
                            )

            for img in range(IMG_PER_CORE):
                h1 = stage1(img)
                if img + 2 < IMG_PER_CORE:
                    xts.append(load_x(img + 2))
                stage2(img, h1)

    _split_multi_waits(nc)
    return nc


_NC_CACHE = None


def kernel(inputs, w0, b0, w1, b1):
    global _NC_CACHE
    x = np.asarray(inputs, dtype=np.float32)
    w0 = np.asarray(w0, dtype=np.float32)
    w1 = np.asarray(w1, dtype=np.float32)
    b0 = np.asarray(b0, dtype=np.float32)
    b1 = np.asarray(b1, dtype=np.float32)

    if _NC_CACHE is None:
        _NC_CACHE = build_nc()
    nc = _NC_CACHE

    bf = ml_dtypes.bfloat16
    # per-core x: [IMG, C, XCOLS] bf16, image transposed to [C, pixels]
    xs = x.reshape(N_CORES, IMG_PER_CORE, HW, C)
    w0h = np.ascontiguousarray(
        w0.reshape(9, C, F).transpose(1, 0, 2).astype(bf)
    )
    w1h = np.ascontiguousarray(
        w1.reshape(2, C, F).transpose(1, 0, 2).astype(bf)
    )
    b0h = np.ascontiguousarray(b0.reshape(2, 128).T)
    b1h = np.ascontiguousarray(b1.reshape(2, 128).T)

    in_maps = []
    for c in range(N_CORES):
        xt = np.zeros((IMG_PER_CORE, C, XCOLS), bf)
        xt[:, :, :HW] = xs[c].transpose(0, 2, 1).astype(bf)
        in_maps.append({"x": xt, "w0": w0h, "w1": w1h, "b0": b0h, "b1": b1h})

    res = run_bass_kernel_spmd(nc, in_maps, core_ids=list(range(N_CORES)))

    final = np.empty((B, 62, 62, F), np.float32)
    vf = final.reshape(F, 62 * 62, B)  # the [F, N, B] view the reference reshapes
    for c in range(N_CORES):
        oc = res.results[c]["out"].reshape(F, IMG_PER_CORE, 62, 64)
        oc = oc[:, :, :, :62].reshape(F, IMG_PER_CORE, 62 * 62)
        for i in range(IMG_PER_CORE):
            vf[:, :, c * IMG_PER_CORE + i] = oc[:, i]
    return final
